# revision 7
# baseline (speedup 1.0000x reference)
"""Trainium2 Bass kernel for nn_AffineTransform (moe_routing).

Reference computation (B=2048, D=256, Z=512, A=64, L=3):
    h        = relu(x @ w1.T + b1)                  # [B, Z]
    s        = softmax(h @ w2.T + b2, axis=-1)      # [B, A]
    out      = x
    for l in range(L):
        M    = (s @ AM[l]).reshape(B, D, D)         # per-sample blended matrix
        off  = s @ AO[l]                            # [B, D]
        out  = einsum('bij,bj->bi', M, out) + off
        if l < L-1: out = prelu(out, a)

Strategy: data-parallel over batch on 8 cores (BS=256/core). Per layer the
heavy contraction is computed expert-first:
    Y_a[b, i] = sum_j O[j, b] * AM_T[l, a, j, i]    (PE, fp32r, psum batch-major)
    acc[b, :] += s[b, a] * Y_a[b, :]                (blend, split DVE/ACT+GPSIMD)
which never materializes the [B, D*D] blended-matrix tensor.

Host-side prep only re-lays-out tensors (transposes/reshapes); all FLOPs of
the reference computation run on device.
"""

import numpy as np

import concourse.bacc as bacc
import concourse.mybir as mybir
import concourse.tile as tile
from concourse import bass_utils
from concourse.masks import make_identity

F32 = mybir.dt.float32
F32R = mybir.dt.float32r
AX = mybir.AxisListType
OP = mybir.AluOpType
AF = mybir.ActivationFunctionType

B, D, Z, A, L = 2048, 256, 512, 64, 3
N_CORES = 8
BS = B // N_CORES          # 256 samples per core
NB = BS // 128             # 2 batch blocks
NJ = D // 128              # 2 contraction blocks
NZ = Z // 128              # 4 router hidden blocks

# --- tunables ---------------------------------------------------------------
DVE_PER_8 = 5              # of every 8 blend units, this many go to DVE-stt
AM_BUFS = 8                # expert weight tiles in flight (256KB each)
Y_BUFS = 6                 # psum banks for expert outputs
MM_R = True                # fp32r for the expert matmuls
DT_MM = F32R if MM_R else F32


def build_nc(alpha: float):
    """Build the per-core Bass program (identical on all cores)."""
    nc = bacc.Bacc("TRN2", target_bir_lowering=False, debug=False,
                   num_devices=N_CORES)

    x_fm = nc.dram_tensor("x_fm", [D, BS], F32, kind="ExternalInput").ap()
    am_t = nc.dram_tensor("am_t", [L, A, D, D], DT_MM,
                          kind="ExternalInput").ap()
    ao = nc.dram_tensor("ao", [L, A, D], F32, kind="ExternalInput").ap()
    w1_t = nc.dram_tensor("w1_t", [D, Z], F32, kind="ExternalInput").ap()
    b1_r = nc.dram_tensor("b1_r", [128, NZ], F32, kind="ExternalInput").ap()
    w2_t = nc.dram_tensor("w2_t", [Z, A], F32, kind="ExternalInput").ap()
    b2_r = nc.dram_tensor("b2_r", [1, A], F32, kind="ExternalInput").ap()
    out = nc.dram_tensor("out", [BS, D], F32, kind="ExternalOutput").ap()

    with tile.TileContext(nc) as tc:
        with (
            tc.tile_pool(name="cpool", bufs=1) as cpool,
            tc.tile_pool(name="ampool", bufs=AM_BUFS) as ampool,
            tc.tile_pool(name="tmppool", bufs=4) as tmppool,
        ):
            # ---- persistent tiles ----
            ident = cpool.tile([128, 128], F32, tag="ident", name="ident")
            make_identity(nc, ident)
            ones1 = cpool.tile([1, 128], F32, tag="ones1", name="ones1")
            nc.gpsimd.memset(ones1, 1.0)

            x_sb = []
            xr_sb = []
            for d in range(NJ):
                t = cpool.tile([128, BS], F32, tag=f"x{d}", name=f"x{d}")
                nc.sync.dma_start(t, x_fm[d * 128:(d + 1) * 128, :])
                x_sb.append(t)
                tr = cpool.tile([128, BS], DT_MM, tag=f"xr{d}", name=f"xr{d}")
                nc.vector.tensor_copy(tr, t)
                xr_sb.append(tr)
            w1_sb = []
            for d in range(NJ):
                t = cpool.tile([128, Z], F32, tag=f"w1_{d}", name=f"w1_{d}")
                nc.sync.dma_start(t, w1_t[d * 128:(d + 1) * 128, :])
                w1_sb.append(t)
            w2_sb = []
            for z in range(NZ):
                t = cpool.tile([128, A], F32, tag=f"w2_{z}", name=f"w2_{z}")
                nc.sync.dma_start(t, w2_t[z * 128:(z + 1) * 128, :])
                w2_sb.append(t)
            b1_sb = cpool.tile([128, NZ], F32, tag="b1", name="b1")
            nc.sync.dma_start(b1_sb, b1_r)
            b2_sb = cpool.tile([1, A], F32, tag="b2", name="b2")
            nc.sync.dma_start(b2_sb, b2_r)

            s_bm = [cpool.tile([128, A], F32, tag=f"s_bm{b}", name=f"s_bm{b}")
                    for b in range(NB)]
            s_fm = cpool.tile([64, BS], F32, tag="s_fm", name="s_fm")
            accD = [cpool.tile([128, D], F32, tag=f"accD{b}", name=f"accD{b}")
                    for b in range(NB)]
            accG = [cpool.tile([128, D], F32, tag=f"accG{b}", name=f"accG{b}")
                    for b in range(NB)]
            # ping-pong activation tiles (feature-major [j, b]) for layers 1,2
            o_pp = [[cpool.tile([128, BS], DT_MM, tag=f"o{p}_{j}",
                                name=f"o{p}_{j}")
                     for j in range(NJ)] for p in range(2)]

            # ---- router ----
            with tc.tile_pool(name="rps", bufs=1, space="PSUM") as rps:
                h_sb = []
                for z in range(NZ):
                    h_ps = rps.tile([128, BS], F32, tag="h", name=f"h_ps{z}",
                                    bufs=NZ)
                    for d in range(NJ):
                        nc.tensor.matmul(h_ps,
                                         w1_sb[d][:, z * 128:(z + 1) * 128],
                                         x_sb[d],
                                         start=(d == 0), stop=(d == NJ - 1))
                    h = tmppool.tile([128, BS], F32, tag=f"h{z}", name=f"h{z}",
                                     bufs=1)
                    nc.scalar.activation(h, h_ps, AF.Relu,
                                         bias=b1_sb[:, z:z + 1], scale=1.0)
                    h_sb.append(h)
                for b in range(NB):
                    lg_ps = rps.tile([128, A], F32, tag="lg", name=f"lg_ps{b}",
                                     bufs=NB)
                    for z in range(NZ):
                        nc.tensor.matmul(lg_ps,
                                         h_sb[z][:, b * 128:(b + 1) * 128],
                                         w2_sb[z],
                                         start=(z == 0), stop=False)
                    nc.tensor.matmul(lg_ps, ones1, b2_sb, start=False, stop=True)
                    negmax = tmppool.tile([128, 1], F32, tag="negmax",
                                          name=f"negmax{b}", bufs=NB)
                    nc.vector.tensor_reduce(negmax, lg_ps, axis=AX.X, op=OP.max,
                                            negate=True)
                    e_sb = tmppool.tile([128, A], F32, tag="e_sb",
                                        name=f"e_sb{b}", bufs=NB)
                    nc.scalar.activation(e_sb, lg_ps, AF.Exp, bias=negmax,
                                         scale=1.0)
                    ssum = tmppool.tile([128, 1], F32, tag="ssum",
                                        name=f"ssum{b}", bufs=NB)
                    nc.vector.tensor_reduce(ssum, e_sb, axis=AX.X, op=OP.add)
                    rinv = tmppool.tile([128, 1], F32, tag="rinv",
                                        name=f"rinv{b}", bufs=NB)
                    nc.vector.reciprocal(rinv, ssum)
                    nc.vector.tensor_scalar(s_bm[b], e_sb, rinv, None,
                                            op0=OP.mult)
                    sT_ps = rps.tile([64, 128], F32, tag="sT", name=f"sT_ps{b}",
                                     bufs=NB)
                    nc.tensor.transpose(sT_ps, s_bm[b], ident)
                    nc.scalar.copy(s_fm[:, b * 128:(b + 1) * 128], sT_ps)

            # ---- layers ----
            o_tiles = xr_sb
            with tc.tile_pool(name="eps", bufs=1, space="PSUM") as eps:
                unit = 0
                for l in range(L):
                    ao_sb = tmppool.tile([64, D], F32, tag="ao", name=f"ao{l}",
                                         bufs=2)
                    nc.sync.dma_start(ao_sb, ao[l])
                    off_ps = eps.tile([128, NB * D], F32, tag="off",
                                      name=f"off{l}", bufs=1)
                    for b in range(NB):
                        nc.tensor.matmul(off_ps[:, b * D:(b + 1) * D],
                                         s_fm[:, b * 128:(b + 1) * 128],
                                         ao_sb, start=True, stop=True)
                        nc.scalar.copy(accD[b], off_ps[:, b * D:(b + 1) * D])
                        nc.gpsimd.memset(accG[b], 0.0)
                    for a in range(A):
                        am_sb = ampool.tile([128, NJ, D], DT_MM, tag="am",
                                            name=f"am{l}_{a}")
                        src = am_t[l, a].rearrange("(jb p) i -> p jb i", p=128)
                        nc.sync.dma_start(am_sb, src)
                        y_ps = eps.tile([128, NB * D], F32, tag="y",
                                        name=f"y{l}_{a}", bufs=Y_BUFS)
                        for b in range(NB):
                            for j in range(NJ):
                                nc.tensor.matmul(
                                    y_ps[:, b * D:(b + 1) * D],
                                    o_tiles[j][:, b * 128:(b + 1) * 128],
                                    am_sb[:, j, :],
                                    start=(j == 0), stop=(j == NJ - 1))
                        for b in range(NB):
                            yv = y_ps[:, b * D:(b + 1) * D]
                            sv = s_bm[b][:, a:a + 1]
                            if (unit % 8) < DVE_PER_8:
                                nc.vector.scalar_tensor_tensor(
                                    accD[b], yv, sv, accD[b],
                                    op0=OP.mult, op1=OP.add)
                            else:
                                tmp = tmppool.tile([128, D], F32, tag="tmp",
                                                   name=f"tmp{l}_{a}_{b}")
                                nc.scalar.activation(tmp, yv, AF.Copy,
                                                     bias=0.0, scale=sv)
                                nc.gpsimd.tensor_tensor(accG[b], accG[b], tmp,
                                                        op=OP.add)
                            unit += 1
                    for b in range(NB):
                        nc.vector.tensor_tensor(accD[b], accD[b], accG[b],
                                                op=OP.add)
                    if l < L - 1:
                        o_next = o_pp[l % 2]
                        for b in range(NB):
                            for jh in range(NJ):
                                tr_ps = eps.tile([128, 128], F32, tag="tr",
                                                 name=f"tr{l}_{b}_{jh}", bufs=1)
                                nc.tensor.transpose(
                                    tr_ps,
                                    accD[b][:, jh * 128:(jh + 1) * 128], ident)
                                dst = o_next[jh][:, b * 128:(b + 1) * 128]
                                # prelu(x) = (1-alpha)*relu(x) + alpha*x
                                nc.scalar.activation(dst, tr_ps, AF.Relu,
                                                     bias=0.0,
                                                     scale=1.0 - alpha)
                                nc.vector.scalar_tensor_tensor(
                                    dst, tr_ps, alpha, dst,
                                    op0=OP.mult, op1=OP.add)
                        o_tiles = o_next
                    else:
                        for b in range(NB):
                            nc.sync.dma_start(out[b * 128:(b + 1) * 128, :],
                                              accD[b])
    nc.compile()
    return nc


_CACHE = {}


def _get_nc(alpha: float):
    key = round(float(alpha), 8)
    if key not in _CACHE:
        _CACHE[key] = build_nc(float(alpha))
    return _CACHE[key]


def _prep_inputs(x, affine_matrices, affine_offsets, w1, b1, w2, b2):
    am_t = np.ascontiguousarray(
        affine_matrices.reshape(L, A, D, D).transpose(0, 1, 3, 2))
    ao = np.ascontiguousarray(affine_offsets)
    w1_t = np.ascontiguousarray(w1.T)
    b1_r = np.ascontiguousarray(b1.reshape(NZ, 128).T)
    w2_t = np.ascontiguousarray(w2.T)
    b2_r = np.ascontiguousarray(b2.reshape(1, A))
    shared = {"am_t": am_t, "ao": ao, "w1_t": w1_t, "b1_r": b1_r,
              "w2_t": w2_t, "b2_r": b2_r}
    in_maps = []
    for c in range(N_CORES):
        xc = np.ascontiguousarray(x[c * BS:(c + 1) * BS].T)  # [D, BS]
        in_maps.append({"x_fm": xc, **shared})
    return in_maps


def run(inputs: dict, trace: bool = False):
    """Run on 8 cores; returns (full_output, BassKernelResults)."""
    inputs = {k: np.asarray(v) for k, v in inputs.items()}
    alpha = float(inputs["prelu_a"])
    nc = _get_nc(alpha)
    in_maps = _prep_inputs(inputs["x"], inputs["affine_matrices"],
                           inputs["affine_offsets"], inputs["w1"],
                           inputs["b1"], inputs["w2"], inputs["b2"])
    res = bass_utils.run_bass_kernel_spmd(
        nc, in_maps, core_ids=list(range(N_CORES)), trace=trace)
    full = np.concatenate([res.results[c]["out"] for c in range(N_CORES)],
                          axis=0)
    return full, res


def kernel(**inputs) -> np.ndarray:
    out, _ = run(inputs, trace=False)
    return out


# revision 10
# speedup vs baseline: 1.2686x; 1.2686x over previous
"""Trainium2 Bass kernel for nn_AffineTransform (moe_routing).

Reference computation (B=2048, D=256, Z=512, A=64, L=3):
    h        = relu(x @ w1.T + b1)                  # [B, Z]
    s        = softmax(h @ w2.T + b2, axis=-1)      # [B, A]
    out      = x
    for l in range(L):
        M    = (s @ AM[l]).reshape(B, D, D)         # per-sample blended matrix
        off  = s @ AO[l]                            # [B, D]
        out  = einsum('bij,bj->bi', M, out) + off
        if l < L-1: out = prelu(out, a)

Strategy: data-parallel over batch on 8 cores (BS=256/core). Per layer the
heavy contraction is computed expert-first:
    Y_a[b, i] = sum_j O[j, b] * AM_T[l, a, j, i]    (PE, fp32r, psum batch-major)
    acc[b, :] += s[b, a] * Y_a[b, :]                (blend, split DVE/ACT+GPSIMD)
which never materializes the [B, D*D] blended-matrix tensor.

Host-side prep only re-lays-out tensors (transposes/reshapes); all FLOPs of
the reference computation run on device.
"""

import numpy as np

import concourse.bacc as bacc
import concourse.mybir as mybir
import concourse.tile as tile
from concourse import bass_utils
from concourse.masks import make_identity

F32 = mybir.dt.float32
F32R = mybir.dt.float32r
AX = mybir.AxisListType
OP = mybir.AluOpType
AF = mybir.ActivationFunctionType

B, D, Z, A, L = 2048, 256, 512, 64, 3
N_CORES = 8
BS = B // N_CORES          # 256 samples per core
NB = BS // 128             # 2 batch blocks
NJ = D // 128              # 2 contraction blocks
NZ = Z // 128              # 4 router hidden blocks

# --- tunables ---------------------------------------------------------------
DVE_PER_8 = 5              # of every 8 blend units, this many go to DVE-stt
AM_BUFS = 8                # expert weight tiles in flight (256KB each)
Y_BUFS = 6                 # psum banks for expert outputs
MM_R = True                # fp32r for the expert matmuls
DT_MM = F32R if MM_R else F32


def build_nc(alpha: float):
    """Build the per-core Bass program (identical on all cores)."""
    nc = bacc.Bacc("TRN2", target_bir_lowering=False, debug=False,
                   num_devices=N_CORES)

    x_fm = nc.dram_tensor("x_fm", [D, BS], F32, kind="ExternalInput").ap()
    am_t = nc.dram_tensor("am_t", [L, A // 2, D, 2 * D], DT_MM,
                          kind="ExternalInput").ap()
    ao = nc.dram_tensor("ao", [L, A, D], F32, kind="ExternalInput").ap()
    w1_t = nc.dram_tensor("w1_t", [D, Z], F32, kind="ExternalInput").ap()
    b1_r = nc.dram_tensor("b1_r", [128, NZ], F32, kind="ExternalInput").ap()
    w2_t = nc.dram_tensor("w2_t", [Z, A], F32, kind="ExternalInput").ap()
    b2_r = nc.dram_tensor("b2_r", [1, A], F32, kind="ExternalInput").ap()
    out = nc.dram_tensor("out", [BS, D], F32, kind="ExternalOutput").ap()

    with tile.TileContext(nc) as tc:
        with (
            tc.tile_pool(name="cpool", bufs=1) as cpool,
            tc.tile_pool(name="ampool", bufs=AM_BUFS) as ampool,
            tc.tile_pool(name="tmppool", bufs=4) as tmppool,
        ):
            # ---- persistent tiles ----
            ident = cpool.tile([128, 128], F32, tag="ident", name="ident")
            make_identity(nc, ident)
            ones1 = cpool.tile([1, 128], F32, tag="ones1", name="ones1")
            nc.gpsimd.memset(ones1, 1.0)

            x_sb = []
            xr_sb = []
            for d in range(NJ):
                t = cpool.tile([128, BS], F32, tag=f"x{d}", name=f"x{d}")
                nc.sync.dma_start(t, x_fm[d * 128:(d + 1) * 128, :])
                x_sb.append(t)
                tr = cpool.tile([128, BS], DT_MM, tag=f"xr{d}", name=f"xr{d}")
                nc.vector.tensor_copy(tr, t)
                xr_sb.append(tr)
            w1_sb = []
            for d in range(NJ):
                t = cpool.tile([128, Z], F32, tag=f"w1_{d}", name=f"w1_{d}")
                nc.sync.dma_start(t, w1_t[d * 128:(d + 1) * 128, :])
                w1_sb.append(t)
            w2_sb = []
            for z in range(NZ):
                t = cpool.tile([128, A], F32, tag=f"w2_{z}", name=f"w2_{z}")
                nc.sync.dma_start(t, w2_t[z * 128:(z + 1) * 128, :])
                w2_sb.append(t)
            b1_sb = cpool.tile([128, NZ], F32, tag="b1", name="b1")
            nc.sync.dma_start(b1_sb, b1_r)
            b2_sb = cpool.tile([1, A], F32, tag="b2", name="b2")
            nc.sync.dma_start(b2_sb, b2_r)

            s_bm = [cpool.tile([128, A], F32, tag=f"s_bm{b}", name=f"s_bm{b}")
                    for b in range(NB)]
            s_fm = cpool.tile([64, BS], F32, tag="s_fm", name="s_fm")
            accD = [cpool.tile([128, D], F32, tag=f"accD{b}", name=f"accD{b}")
                    for b in range(NB)]
            accG = [cpool.tile([128, D], F32, tag=f"accG{b}", name=f"accG{b}")
                    for b in range(NB)]
            # ping-pong activation tiles (feature-major [j, b]) for layers 1,2
            o_pp = [[cpool.tile([128, BS], DT_MM, tag=f"o{p}_{j}",
                                name=f"o{p}_{j}")
                     for j in range(NJ)] for p in range(2)]

            # ---- router ----
            with tc.tile_pool(name="rps", bufs=1, space="PSUM") as rps:
                h_sb = []
                for z in range(NZ):
                    h_ps = rps.tile([128, BS], F32, tag="h", name=f"h_ps{z}",
                                    bufs=NZ)
                    for d in range(NJ):
                        nc.tensor.matmul(h_ps,
                                         w1_sb[d][:, z * 128:(z + 1) * 128],
                                         x_sb[d],
                                         start=(d == 0), stop=(d == NJ - 1))
                    h = tmppool.tile([128, BS], F32, tag=f"h{z}", name=f"h{z}",
                                     bufs=1)
                    nc.scalar.activation(h, h_ps, AF.Relu,
                                         bias=b1_sb[:, z:z + 1], scale=1.0)
                    h_sb.append(h)
                for b in range(NB):
                    lg_ps = rps.tile([128, A], F32, tag="lg", name=f"lg_ps{b}",
                                     bufs=NB)
                    for z in range(NZ):
                        nc.tensor.matmul(lg_ps,
                                         h_sb[z][:, b * 128:(b + 1) * 128],
                                         w2_sb[z],
                                         start=(z == 0), stop=False)
                    nc.tensor.matmul(lg_ps, ones1, b2_sb, start=False, stop=True)
                    negmax = tmppool.tile([128, 1], F32, tag="negmax",
                                          name=f"negmax{b}", bufs=NB)
                    nc.vector.tensor_reduce(negmax, lg_ps, axis=AX.X, op=OP.max,
                                            negate=True)
                    e_sb = tmppool.tile([128, A], F32, tag="e_sb",
                                        name=f"e_sb{b}", bufs=NB)
                    nc.scalar.activation(e_sb, lg_ps, AF.Exp, bias=negmax,
                                         scale=1.0)
                    ssum = tmppool.tile([128, 1], F32, tag="ssum",
                                        name=f"ssum{b}", bufs=NB)
                    nc.vector.tensor_reduce(ssum, e_sb, axis=AX.X, op=OP.add)
                    rinv = tmppool.tile([128, 1], F32, tag="rinv",
                                        name=f"rinv{b}", bufs=NB)
                    nc.vector.reciprocal(rinv, ssum)
                    nc.vector.tensor_scalar(s_bm[b], e_sb, rinv, None,
                                            op0=OP.mult)
                    sT_ps = rps.tile([64, 128], F32, tag="sT", name=f"sT_ps{b}",
                                     bufs=NB)
                    nc.tensor.transpose(sT_ps, s_bm[b], ident)
                    nc.scalar.copy(s_fm[:, b * 128:(b + 1) * 128], sT_ps)

            # ---- layers ----
            o_tiles = xr_sb
            with tc.tile_pool(name="eps", bufs=1, space="PSUM") as eps:
                unit = 0
                for l in range(L):
                    ao_sb = tmppool.tile([64, D], F32, tag="ao", name=f"ao{l}",
                                         bufs=2)
                    nc.sync.dma_start(ao_sb, ao[l])
                    off_ps = eps.tile([128, NB * D], F32, tag="off",
                                      name=f"off{l}", bufs=1)
                    for b in range(NB):
                        nc.tensor.matmul(off_ps[:, b * D:(b + 1) * D],
                                         s_fm[:, b * 128:(b + 1) * 128],
                                         ao_sb, start=True, stop=True)
                        nc.scalar.copy(accD[b], off_ps[:, b * D:(b + 1) * D])
                        nc.gpsimd.memset(accG[b], 0.0)
                    for a0 in range(0, A, 2):
                        # two experts per matmul: N=512 amortizes the fp32r
                        # fused weight load over twice the moving columns
                        am_sb = ampool.tile([128, NJ, 2 * D], DT_MM,
                                            tag="am", name=f"am{l}_{a0}")
                        src = am_t[l, a0 // 2].rearrange(
                            "(jb p) i -> p jb i", p=128)
                        nc.sync.dma_start(am_sb, src)
                        for b in range(NB):
                            y_ps = eps.tile([128, 2 * D], F32, tag="y",
                                            name=f"y{l}_{a0}_{b}", bufs=Y_BUFS)
                            for j in range(NJ):
                                nc.tensor.matmul(
                                    y_ps,
                                    o_tiles[j][:, b * 128:(b + 1) * 128],
                                    am_sb[:, j, :],
                                    start=(j == 0), stop=(j == NJ - 1))
                            for ah in range(2):
                                yv = y_ps[:, ah * D:(ah + 1) * D]
                                sv = s_bm[b][:, a0 + ah:a0 + ah + 1]
                                if (unit % 8) < DVE_PER_8:
                                    nc.vector.scalar_tensor_tensor(
                                        accD[b], yv, sv, accD[b],
                                        op0=OP.mult, op1=OP.add)
                                else:
                                    tmp = tmppool.tile(
                                        [128, D], F32, tag="tmp",
                                        name=f"tmp{l}_{a0}_{b}_{ah}")
                                    nc.scalar.activation(tmp, yv, AF.Copy,
                                                         bias=0.0, scale=sv)
                                    nc.gpsimd.tensor_tensor(accG[b], accG[b],
                                                            tmp, op=OP.add)
                                unit += 1
                    for b in range(NB):
                        nc.vector.tensor_tensor(accD[b], accD[b], accG[b],
                                                op=OP.add)
                    if l < L - 1:
                        o_next = o_pp[l % 2]
                        for b in range(NB):
                            for jh in range(NJ):
                                tr_ps = eps.tile([128, 128], F32, tag="tr",
                                                 name=f"tr{l}_{b}_{jh}", bufs=1)
                                nc.tensor.transpose(
                                    tr_ps,
                                    accD[b][:, jh * 128:(jh + 1) * 128], ident)
                                dst = o_next[jh][:, b * 128:(b + 1) * 128]
                                # prelu(x) = (1-alpha)*relu(x) + alpha*x
                                nc.scalar.activation(dst, tr_ps, AF.Relu,
                                                     bias=0.0,
                                                     scale=1.0 - alpha)
                                nc.vector.scalar_tensor_tensor(
                                    dst, tr_ps, alpha, dst,
                                    op0=OP.mult, op1=OP.add)
                        o_tiles = o_next
                    else:
                        for b in range(NB):
                            nc.sync.dma_start(out[b * 128:(b + 1) * 128, :],
                                              accD[b])
    nc.compile()
    return nc


_CACHE = {}


def _get_nc(alpha: float):
    key = round(float(alpha), 8)
    if key not in _CACHE:
        _CACHE[key] = build_nc(float(alpha))
    return _CACHE[key]


def _prep_inputs(x, affine_matrices, affine_offsets, w1, b1, w2, b2):
    am_jab = affine_matrices.reshape(L, A // 2, 2, D, D)
    am_t = np.ascontiguousarray(
        am_jab.transpose(0, 1, 4, 2, 3)).reshape(L, A // 2, D, 2 * D)
    ao = np.ascontiguousarray(affine_offsets)
    w1_t = np.ascontiguousarray(w1.T)
    b1_r = np.ascontiguousarray(b1.reshape(NZ, 128).T)
    w2_t = np.ascontiguousarray(w2.T)
    b2_r = np.ascontiguousarray(b2.reshape(1, A))
    shared = {"am_t": am_t, "ao": ao, "w1_t": w1_t, "b1_r": b1_r,
              "w2_t": w2_t, "b2_r": b2_r}
    in_maps = []
    for c in range(N_CORES):
        xc = np.ascontiguousarray(x[c * BS:(c + 1) * BS].T)  # [D, BS]
        in_maps.append({"x_fm": xc, **shared})
    return in_maps


def run(inputs: dict, trace: bool = False):
    """Run on 8 cores; returns (full_output, BassKernelResults)."""
    inputs = {k: np.asarray(v) for k, v in inputs.items()}
    alpha = float(inputs["prelu_a"])
    nc = _get_nc(alpha)
    in_maps = _prep_inputs(inputs["x"], inputs["affine_matrices"],
                           inputs["affine_offsets"], inputs["w1"],
                           inputs["b1"], inputs["w2"], inputs["b2"])
    res = bass_utils.run_bass_kernel_spmd(
        nc, in_maps, core_ids=list(range(N_CORES)), trace=trace)
    full = np.concatenate([res.results[c]["out"] for c in range(N_CORES)],
                          axis=0)
    return full, res


def kernel(**inputs) -> np.ndarray:
    out, _ = run(inputs, trace=False)
    return out
